# revision 12
# baseline (speedup 1.0000x reference)
"""Trainium2 Bass kernel for nn_ArrdecoderModel (attention + 2-layer LSTM decoder step).

Data-parallel over batch: B=64 split across 8 NeuronCores (8 per core), weights
replicated, no collectives. Per core the dominant cost is streaming the
(8, 4096, 128) f32 encoder_output shard (16 MiB) from HBM; it is cast to bf16
during the DMA (SWDGE). Word layout is p-major (w = 32*p + k) so the softmax's
one-word shift is a free-dim offset; the 128 partition-boundary words come from
one shift-matrix matmul. Scores are a DVE multiply + pairwise tree + reduce;
the weighted sum runs on the PE with the exp-weight column stationary. The
LSTM tail runs in feature-major layout with bf16 weights.
"""

import numpy as np

import concourse.bass as bass
import concourse.tile as tile
from concourse import bacc, mybir
from concourse.bass_utils import run_bass_kernel_spmd
from concourse.masks import make_identity

F32 = mybir.dt.float32
BF16 = mybir.dt.bfloat16

N_CORES = 8
B = 64
BL = B // N_CORES  # 8 batches per core
W = 4096
H = 128
I = 128
KC = 32            # w = 32*p + k
P = 128

MUL = mybir.AluOpType.mult
ADD = mybir.AluOpType.add
AF = mybir.ActivationFunctionType


def build_bass(debug: bool = False) -> bass.Bass:
    nc = bacc.Bacc("TRN2", target_bir_lowering=False, debug=debug)

    inp = nc.declare_dram_parameter("input", [BL, 1, I], F32, isOutput=False)
    h0 = nc.declare_dram_parameter("h0", [2, BL, H], F32, isOutput=False)
    c0 = nc.declare_dram_parameter("c0", [2, BL, H], F32, isOutput=False)
    enc = nc.declare_dram_parameter("encoder_output", [BL, W, H], F32, isOutput=False)
    att_W = nc.declare_dram_parameter("att_W", [1, 2 * H], F32, isOutput=False)
    att_b = nc.declare_dram_parameter("att_b", [1], F32, isOutput=False)
    inp_W = nc.declare_dram_parameter("inp_W", [I, H + I], F32, isOutput=False)
    inp_b = nc.declare_dram_parameter("inp_b", [I], F32, isOutput=False)
    W_ih = [
        nc.declare_dram_parameter("W_ih0", [4 * H, I], F32, isOutput=False),
        nc.declare_dram_parameter("W_ih1", [4 * H, H], F32, isOutput=False),
    ]
    W_hh = [
        nc.declare_dram_parameter("W_hh0", [4 * H, H], F32, isOutput=False),
        nc.declare_dram_parameter("W_hh1", [4 * H, H], F32, isOutput=False),
    ]
    b_ih = [
        nc.declare_dram_parameter("b_ih0", [4 * H], F32, isOutput=False),
        nc.declare_dram_parameter("b_ih1", [4 * H], F32, isOutput=False),
    ]
    b_hh = [
        nc.declare_dram_parameter("b_hh0", [4 * H], F32, isOutput=False),
        nc.declare_dram_parameter("b_hh1", [4 * H], F32, isOutput=False),
    ]
    # packed output: [h1, h2, c1, c2] each (BL, H)
    out = nc.declare_dram_parameter("out", [4, BL, H], F32, isOutput=True)

    with tile.TileContext(nc) as tc:
        with (
            tc.tile_pool(name="const", bufs=1) as cpool,
            tc.tile_pool(name="enc", bufs=BL) as epool,
            tc.tile_pool(name="work", bufs=3) as wpool,
            tc.tile_pool(name="small", bufs=4) as spool,
            tc.tile_pool(name="psum", bufs=3, space="PSUM") as ppool,
            tc.tile_pool(name="psumx", bufs=2, space="PSUM") as pxpool,
        ):
            # ---------------- gpsimd constants (before any SWDGE DMA) ----------------
            identity = cpool.tile([P, P], F32)
            make_identity(nc, identity[:])
            # shiftmat[k, m] = 1 iff m == k+1  ->  (lhsT=shiftmat): out[m] = rhs[m-1]
            shiftmat = cpool.tile([P, P], F32)
            nc.gpsimd.memset(shiftmat[:], 0.0)
            nc.gpsimd.affine_select(
                out=shiftmat[:], in_=shiftmat[:],
                compare_op=mybir.AluOpType.not_equal,
                fill=1.0, base=1, pattern=[[-1, P]], channel_multiplier=1,
            )

            # ---------------- encoder stream (SWDGE FIFO: in batch order) -------------
            # batch 7 split into two halves, second half first, to shorten the
            # last-arriving dependency chain.
            enc_sb = []
            w_nats = {}
            ipw_nat = None
            for b in range(BL):
                t = epool.tile([P, KC, H], BF16, tag="enc", name=f"enc_sb{b}")
                if b == BL - 1:
                    esrc = enc[b].rearrange("(p k) h -> p k h", k=KC)
                    Q = KC // 4
                    for qi in (3, 2, 1, 0):
                        nc.gpsimd.dma_start(
                            t[:, qi * Q : (qi + 1) * Q, :], esrc[:, qi * Q : (qi + 1) * Q, :]
                        )
                else:
                    nc.gpsimd.dma_start(t[:], enc[b].rearrange("(p k) h -> p k h", k=KC))
                enc_sb.append(t)
                if b == 1:
                    # weight loads ride the SWDGE FIFO behind the first two batches
                    for name, param in (
                        ("ih0", W_ih[0]), ("hh0", W_hh[0]),
                        ("ih1", W_ih[1]), ("hh1", W_hh[1]),
                    ):
                        nat = wpool.tile([P, 4, H], BF16, tag=f"wnat_{name}", bufs=1, name=f"wnat_{name}")
                        nc.gpsimd.dma_start(nat[:], param.rearrange("(g p) c -> p g c", p=P))
                        w_nats[name] = nat
                    ipw_nat = wpool.tile([P, 2, H], BF16, tag="wnat_ipw", bufs=1)
                    nc.gpsimd.dma_start(ipw_nat[:], inp_W.rearrange("m (j c) -> m j c", c=H))

            # ---------------- small loads (HWDGE sync queue; early FIFO slots) -------
            ones_row = cpool.tile([1, P], F32)
            nc.vector.memset(ones_row[:], 1.0)
            ones_col = cpool.tile([P, 1], F32)
            nc.vector.memset(ones_col[:], 1.0)
            onesb_row = cpool.tile([1, BL], BF16)
            nc.vector.memset(onesb_row[:], 1.0)

            # pre-warm ACT function tables so no table load lands mid-pipeline
            warm = cpool.tile([1, 1], F32)
            nc.scalar.activation(warm[:], ones_row[0:1, 0:1], AF.Exp)
            nc.scalar.activation(warm[:], ones_row[0:1, 0:1], AF.Sigmoid)
            nc.scalar.activation(warm[:], ones_row[0:1, 0:1], AF.Tanh)

            attW_row = cpool.tile([1, 2 * H], F32)
            nc.sync.dma_start(attW_row[:], att_W[:])
            attb_t = cpool.tile([1, 1], F32)
            nc.sync.dma_start(attb_t[:], att_b[:])
            h0_nat = cpool.tile([BL, 2, H], F32)
            nc.sync.dma_start(h0_nat[:], h0.rearrange("l b h -> b l h"))
            c0_nat = cpool.tile([BL, 2, H], F32)
            nc.sync.dma_start(c0_nat[:], c0.rearrange("l b h -> b l h"))
            in_nat = cpool.tile([BL, I], F32)
            nc.sync.dma_start(in_nat[:], inp[:, 0, :])
            b_rows = []
            for l in range(2):
                bi = cpool.tile([1, 4 * H], F32, tag=f"bi{l}")
                nc.sync.dma_start(bi[:], b_ih[l][:])
                bh = cpool.tile([1, 4 * H], F32, tag=f"bh{l}")
                nc.sync.dma_start(bh[:], b_hh[l][:])
                b_rows.append((bi, bh))
            ipb_row = cpool.tile([1, I], F32)
            nc.sync.dma_start(ipb_row[:], inp_b[:])

            # ---------------- early prep: attention bias, state transposes ----------
            identity_bf = cpool.tile([P, P], BF16)
            nc.vector.tensor_copy(identity_bf[:], identity[:])

            wa_ps = ppool.tile([P, H], F32, tag="ps")
            nc.tensor.matmul(wa_ps[:], ones_row[:], attW_row[:, 0:H], start=True, stop=True)
            wa_bc = cpool.tile([P, H], BF16)
            nc.scalar.copy(wa_bc[:], wa_ps[:])

            wst_ps = ppool.tile([P, 1], F32, tag="ps")
            nc.tensor.matmul(wst_ps[:], attW_row[:, H : 2 * H], ones_col[0:1, :], start=True, stop=True)
            wst_col = cpool.tile([P, 1], F32)
            nc.scalar.copy(wst_col[:], wst_ps[:])

            h0T = cpool.tile([P, 2, BL], F32)
            h0Tb = cpool.tile([P, 2, BL], BF16)
            c0T = cpool.tile([P, 2, BL], F32)
            for l in range(2):
                pt = ppool.tile([P, BL], F32, tag="ps")
                nc.tensor.transpose(pt[:], h0_nat[:, l, :], identity[0:BL, 0:BL])
                nc.scalar.copy(h0T[:, l, :], pt[:])
                nc.vector.tensor_copy(h0Tb[:, l, :], pt[:])
                pt2 = ppool.tile([P, BL], F32, tag="ps")
                nc.tensor.transpose(pt2[:], c0_nat[:, l, :], identity[0:BL, 0:BL])
                nc.scalar.copy(c0T[:, l, :], pt2[:])
            inT_ps = ppool.tile([P, BL], F32, tag="ps")
            nc.tensor.transpose(inT_ps[:], in_nat[:], identity[0:BL, 0:BL])
            inputT = cpool.tile([P, BL], BF16)
            nc.scalar.copy(inputT[:], inT_ps[:])

            astT = cpool.tile([P, BL], F32)
            nc.vector.tensor_tensor(astT[:], h0T[:, 1, :], c0T[:, 1, :], MUL)
            sb_ps = ppool.tile([1, BL], F32, tag="ps")
            nc.tensor.matmul(sb_ps[:], wst_col[:], astT[:], start=True, stop=True)
            sb_row = cpool.tile([1, BL], F32)
            nc.scalar.activation(sb_row[:], sb_ps[:], AF.Identity, bias=attb_t[0:1, 0:1])
            ba_ps = ppool.tile([P, BL], F32, tag="ps")
            nc.tensor.matmul(ba_ps[:], ones_row[:], sb_row[:], start=True, stop=True)
            bias_all = cpool.tile([P, BL], F32)
            nc.scalar.copy(bias_all[:], ba_ps[:])

            # ---------------- weight prep (weights arrive early via SWDGE head) -------
            wT = {}
            for name, nat in w_nats.items():
                dst = cpool.tile([P, 4, H], BF16, tag=f"wT_{name}", name=f"wT_{name}")
                for g in range(4):
                    pt = ppool.tile([P, P], BF16, tag="psb", bufs=2)
                    nc.tensor.transpose(pt[:], nat[:, g, :], identity_bf[:])
                    nc.scalar.copy(dst[:, g, :], pt[:])
                wT[name] = dst
            inp_WT = cpool.tile([P, 2, H], BF16)
            for j in range(2):
                pt = ppool.tile([P, P], BF16, tag="psb", bufs=2)
                nc.tensor.transpose(pt[:], ipw_nat[:, j, :], identity_bf[:])
                nc.scalar.copy(inp_WT[:, j, :], pt[:])

            bsum_rows = []
            for l in range(2):
                bi, bh = b_rows[l]
                bsum = cpool.tile([1, 4 * H], BF16, tag=f"bsum{l}", name=f"bsum{l}")
                nc.vector.tensor_tensor(bsum[:], bi[:], bh[:], ADD)
                bsum_rows.append(bsum)
            ipb_ps = ppool.tile([P, 1], F32, tag="ps")
            nc.tensor.matmul(ipb_ps[:], ipb_row[:], ones_col[0:1, :], start=True, stop=True)
            ipb_col = cpool.tile([P, 1], F32)
            nc.scalar.copy(ipb_col[:], ipb_ps[:])

            # ---------------- main loop over local batches ----------------
            xT_sb = cpool.tile([P, BL], BF16)

            def score_half(b, tag, k0, k1):
                """scores for word columns [k0, k1) of batch b (f32)."""
                kw = k1 - k0
                tmp = wpool.tile([P, kw, H], BF16, tag=f"tmp{tag}", name=f"tmp{tag}_{b}")
                nc.vector.tensor_tensor(
                    tmp[:], enc_sb[b][:, k0:k1, :],
                    wa_bc[:, None, :].to_broadcast((P, kw, H)), MUL,
                )
                t1 = wpool.tile([P, kw, 64], BF16, tag=f"t1{tag}", name=f"t1{tag}_{b}")
                nc.vector.tensor_tensor(t1[:], tmp[:, :, 0:64], tmp[:, :, 64:128], ADD)
                t2 = wpool.tile([P, kw, 32], BF16, tag=f"t2{tag}", name=f"t2{tag}_{b}")
                nc.vector.tensor_tensor(t2[:], t1[:, :, 0:32], t1[:, :, 32:64], ADD)
                t3 = wpool.tile([P, kw, 16], BF16, tag=f"t3{tag}", name=f"t3{tag}_{b}")
                nc.vector.tensor_tensor(t3[:], t2[:, :, 0:16], t2[:, :, 16:32], ADD)
                s_out = spool.tile([P, kw], F32, tag=f"s{tag}", name=f"s{tag}_{b}")
                nc.vector.tensor_reduce(s_out[:], t3[:], mybir.AxisListType.X, ADD)
                return s_out

            for b in range(BL):
                e_sb = spool.tile([P, KC], BF16, tag="e", name=f"e_sb{b}")
                bias_col = bias_all[:, b : b + 1]

                if b == BL - 1:
                    # quarters in DMA-arrival order: k 24..31 first, 0..7 last
                    Q = KC // 4
                    accs = []
                    for j, qi in enumerate((3, 2, 1, 0)):
                        s_q = score_half(b, f"q{qi}", qi * Q, (qi + 1) * Q)
                        if qi == 3:
                            bnd_ps = ppool.tile([P, 1], F32, tag="ps")
                            nc.tensor.matmul(bnd_ps[:], shiftmat[:], s_q[:, -1:], start=True, stop=True)
                            nc.scalar.activation(e_sb[:, 0:1], bnd_ps[:], AF.Exp, bias=bias_col)
                            hi = KC - 1
                        else:
                            hi = (qi + 1) * Q
                        acc = spool.tile([P, 1], F32, tag=f"acc{qi}", name=f"acc{qi}")
                        # e[:, k+1] = exp(s[:, k]) for k in this quarter (minus the last word overall)
                        nc.scalar.activation(
                            e_sb[:, qi * Q + 1 : hi + 1], s_q[:, 0 : hi - qi * Q], AF.Exp,
                            bias=bias_col, accum_out=acc[:],
                        )
                        accs.append(acc)
                else:
                    s_sb = score_half(b, "f", 0, KC)
                    bnd_ps = ppool.tile([P, 1], F32, tag="ps")
                    nc.tensor.matmul(bnd_ps[:], shiftmat[:], s_sb[:, KC - 1 : KC], start=True, stop=True)
                    acc1 = spool.tile([P, 1], F32, tag="acc1")
                    nc.scalar.activation(
                        e_sb[:, 1:KC], s_sb[:, 0 : KC - 1], AF.Exp, bias=bias_col,
                        accum_out=acc1[:],
                    )
                    nc.scalar.activation(e_sb[:, 0:1], bnd_ps[:], AF.Exp, bias=bias_col)
                nc.vector.memset(e_sb[0:1, 0:1], 1.0)

                # Z = sum(e): accumulated exp sums + the word-0 column
                esum = spool.tile([P, 1], F32, tag="esum")
                if b == BL - 1:
                    nc.vector.tensor_tensor(esum[:], accs[0][:], accs[1][:], ADD)
                    nc.vector.tensor_tensor(esum[:], esum[:], accs[2][:], ADD)
                    nc.vector.tensor_tensor(esum[:], esum[:], accs[3][:], ADD)
                    nc.vector.tensor_tensor(esum[:], esum[:], e_sb[:, 0:1], ADD)
                else:
                    nc.vector.tensor_tensor(esum[:], acc1[:], e_sb[:, 0:1], ADD)
                z_ps = ppool.tile([1, 1], F32, tag="ps")
                nc.tensor.matmul(z_ps[:], esum[:], ones_col[:], start=True, stop=True)
                rz = spool.tile([1, 1], F32, tag="rz")
                nc.vector.reciprocal(rz[:], z_ps[:])
                rz_ps = ppool.tile([P, 1], F32, tag="ps")
                nc.tensor.matmul(rz_ps[:], ones_row[:], rz[:], start=True, stop=True)
                rz_col = spool.tile([P, 1], F32, tag="rzc")
                nc.scalar.copy(rz_col[:], rz_ps[:])

                # weighted sum: x row = sum_k e_k.T @ enc_chunk_k (e stationary)
                xrow_ps = pxpool.tile([1, P], F32, tag="xr")
                if b == BL - 1:
                    Q = KC // 4
                    korder = []
                    for qi in (3, 2, 1, 0):
                        lo = qi * Q + (1 if qi == 0 else 0)
                        korder += list(range(lo, (qi + 1) * Q))
                    korder.append(0)
                else:
                    korder = list(range(KC))
                for j, k in enumerate(korder):
                    nc.tensor.matmul(
                        xrow_ps[:], e_sb[:, k : k + 1], enc_sb[b][:, k, :],
                        start=(j == 0), stop=(j == KC - 1),
                    )
                x_row = spool.tile([1, P], F32, tag="xrow")
                nc.scalar.copy(x_row[:], xrow_ps[:])
                xcol = pxpool.tile([P, 1], F32, tag="xc", bufs=1)
                nc.tensor.matmul(xcol[:], x_row[:], ones_col[0:1, :], start=True, stop=True)
                nc.vector.tensor_tensor(xT_sb[:, b : b + 1], xcol[:], rz_col[:], MUL)

            # ---------------- tail: input MLP + 2-layer LSTM (bf16 matmuls) ---------
            xin_ps = ppool.tile([P, BL], F32, tag="ps")
            nc.tensor.matmul(xin_ps[:], inp_WT[:, 0, :], xT_sb[:], start=True, stop=False)
            nc.tensor.matmul(xin_ps[:], inp_WT[:, 1, :], inputT[:], start=False, stop=True)
            xinT = cpool.tile([P, BL], BF16)
            nc.scalar.activation(xinT[:], xin_ps[:], AF.Identity, bias=ipb_col[:])

            hcT = cpool.tile([P, 4 * BL], F32)  # cols: h1 | h2 | c1 | c2
            h1Tb = cpool.tile([P, BL], BF16)

            # psum gate column order [i, f, o, g]; weight row-chunks are (i,f,g,o)
            GATE_ORDER = (0, 1, 3, 2)
            for l in range(2):
                x_rhs = xinT[:] if l == 0 else h1Tb[:]
                wih, whh = wT[f"ih{l}"], wT[f"hh{l}"]
                bsum = bsum_rows[l]
                g_ps = ppool.tile([P, 4 * BL], F32, tag="ps")
                for col, g in enumerate(GATE_ORDER):
                    sl = g_ps[:, col * BL : (col + 1) * BL]
                    nc.tensor.matmul(sl, wih[:, g, :], x_rhs, start=True, stop=False)
                    nc.tensor.matmul(sl, whh[:, g, :], h0Tb[:, l, :], start=False, stop=False)
                    nc.tensor.matmul(
                        sl, bsum[:, g * H : (g + 1) * H], onesb_row[:], start=False, stop=True
                    )
                sig = spool.tile([P, 3 * BL], F32, tag="sig")
                nc.scalar.activation(sig[:], g_ps[:, 0 : 3 * BL], AF.Sigmoid)
                tng = spool.tile([P, BL], F32, tag="tng")
                nc.scalar.activation(tng[:], g_ps[:, 3 * BL : 4 * BL], AF.Tanh)

                fc = spool.tile([P, BL], F32, tag="fc")
                nc.vector.tensor_tensor(fc[:], sig[:, BL : 2 * BL], c0T[:, l, :], MUL)
                ig = spool.tile([P, BL], F32, tag="ig")
                nc.vector.tensor_tensor(ig[:], sig[:, 0:BL], tng[:], MUL)
                c_new = hcT[:, (2 + l) * BL : (3 + l) * BL]
                nc.vector.tensor_tensor(c_new, fc[:], ig[:], ADD)
                tc_t = spool.tile([P, BL], F32, tag="tc")
                nc.scalar.activation(tc_t[:], c_new, AF.Tanh)
                h_new = hcT[:, l * BL : (l + 1) * BL]
                nc.vector.tensor_tensor(h_new, sig[:, 2 * BL : 3 * BL], tc_t[:], MUL)
                if l == 0:
                    nc.vector.tensor_copy(h1Tb[:], h_new)

            outT_ps = ppool.tile([4 * BL, P], F32, tag="ps")
            nc.tensor.transpose(outT_ps[:], hcT[:], identity[:])
            out_stage = cpool.tile([4 * BL, P], F32)
            nc.scalar.copy(out_stage[:], outT_ps[:])
            nc.sync.dma_start(out.rearrange("j b h -> (j b) h"), out_stage[:])

    nc.compile()
    return nc


_NC_CACHE = {}


def _get_nc():
    if "nc" not in _NC_CACHE:
        _NC_CACHE["nc"] = build_bass(debug=False)
    return _NC_CACHE["nc"]


def _shard_inputs(inputs: dict) -> list[dict]:
    in_maps = []
    for i in range(N_CORES):
        sl = slice(i * BL, (i + 1) * BL)
        m = {
            "input": np.ascontiguousarray(inputs["input"][sl]),
            "h0": np.ascontiguousarray(inputs["h0"][:, sl]),
            "c0": np.ascontiguousarray(inputs["c0"][:, sl]),
            "encoder_output": np.ascontiguousarray(inputs["encoder_output"][sl]),
        }
        for k in (
            "att_W", "att_b", "inp_W", "inp_b",
            "W_ih0", "W_hh0", "b_ih0", "b_hh0",
            "W_ih1", "W_hh1", "b_ih1", "b_hh1",
        ):
            m[k] = np.ascontiguousarray(inputs[k])
        in_maps.append(m)
    return in_maps


def run_spmd(inputs: dict, trace: bool = False):
    """Returns (outputs_tuple, exec_time_ns_or_None)."""
    nc = _get_nc()
    in_maps = _shard_inputs(inputs)
    res = run_bass_kernel_spmd(nc, in_maps, list(range(N_CORES)), trace=trace)
    packed = np.concatenate([res.results[i]["out"] for i in range(N_CORES)], axis=1)
    h1, h2, c1, c2 = packed[0], packed[1], packed[2], packed[3]
    output = h2[:, None, :].astype(np.float32)
    h_stack = np.stack([h1, h2]).astype(np.float32)
    c_stack = np.stack([c1, c2]).astype(np.float32)
    return (output, h_stack, c_stack), res.exec_time_ns


def kernel(**inputs):
    inputs = {k: np.asarray(v, dtype=np.float32) for k, v in inputs.items()}
    outs, _ = run_spmd(inputs, trace=False)
    return outs


# revision 13
# speedup vs baseline: 1.2387x; 1.2387x over previous
"""Trainium2 Bass kernel for nn_ArrdecoderModel (attention + 2-layer LSTM decoder step).

Data-parallel over batch: B=64 split across 8 NeuronCores (8 per core), weights
replicated, no collectives. Per core the dominant cost is streaming the
(8, 4096, 128) f32 encoder_output shard (16 MiB) from HBM; it is cast to bf16
during the DMA (SWDGE). Word layout is p-major (w = 32*p + k) so the softmax's
one-word shift is a free-dim offset; the 128 partition-boundary words come from
one shift-matrix matmul. Scores are a DVE multiply + pairwise tree + reduce;
the weighted sum runs on the PE with the exp-weight column stationary. The
LSTM tail runs in feature-major layout with bf16 weights.
"""

import numpy as np

import concourse.bass as bass
import concourse.tile as tile
from concourse import bacc, mybir
from concourse.bass_utils import run_bass_kernel_spmd
from concourse.masks import make_identity

F32 = mybir.dt.float32
BF16 = mybir.dt.bfloat16

N_CORES = 8
B = 64
BL = B // N_CORES  # 8 batches per core
W = 4096
H = 128
I = 128
KC = 32            # w = 32*p + k
P = 128

MUL = mybir.AluOpType.mult
ADD = mybir.AluOpType.add
AF = mybir.ActivationFunctionType


def build_bass(debug: bool = False) -> bass.Bass:
    nc = bacc.Bacc("TRN2", target_bir_lowering=False, debug=debug)

    inp = nc.declare_dram_parameter("input", [BL, 1, I], F32, isOutput=False)
    h0 = nc.declare_dram_parameter("h0", [2, BL, H], F32, isOutput=False)
    c0 = nc.declare_dram_parameter("c0", [2, BL, H], F32, isOutput=False)
    enc = nc.declare_dram_parameter("encoder_output", [BL, W, H], F32, isOutput=False)
    att_W = nc.declare_dram_parameter("att_W", [1, 2 * H], F32, isOutput=False)
    att_b = nc.declare_dram_parameter("att_b", [1], F32, isOutput=False)
    inp_W = nc.declare_dram_parameter("inp_W", [I, H + I], F32, isOutput=False)
    inp_b = nc.declare_dram_parameter("inp_b", [I], F32, isOutput=False)
    W_ih = [
        nc.declare_dram_parameter("W_ih0", [4 * H, I], F32, isOutput=False),
        nc.declare_dram_parameter("W_ih1", [4 * H, H], F32, isOutput=False),
    ]
    W_hh = [
        nc.declare_dram_parameter("W_hh0", [4 * H, H], F32, isOutput=False),
        nc.declare_dram_parameter("W_hh1", [4 * H, H], F32, isOutput=False),
    ]
    b_ih = [
        nc.declare_dram_parameter("b_ih0", [4 * H], F32, isOutput=False),
        nc.declare_dram_parameter("b_ih1", [4 * H], F32, isOutput=False),
    ]
    b_hh = [
        nc.declare_dram_parameter("b_hh0", [4 * H], F32, isOutput=False),
        nc.declare_dram_parameter("b_hh1", [4 * H], F32, isOutput=False),
    ]
    # packed output: [h1, h2, c1, c2] each (BL, H)
    out = nc.declare_dram_parameter("out", [4, BL, H], F32, isOutput=True)

    with tile.TileContext(nc) as tc:
        with (
            tc.tile_pool(name="const", bufs=1) as cpool,
            tc.tile_pool(name="enc", bufs=BL) as epool,
            tc.tile_pool(name="work", bufs=3) as wpool,
            tc.tile_pool(name="small", bufs=4) as spool,
            tc.tile_pool(name="psum", bufs=3, space="PSUM") as ppool,
            tc.tile_pool(name="psumx", bufs=2, space="PSUM") as pxpool,
        ):
            # ---------------- gpsimd constants (before any SWDGE DMA) ----------------
            identity = cpool.tile([P, P], F32)
            make_identity(nc, identity[:])
            # shiftmat[k, m] = 1 iff m == k+1  ->  (lhsT=shiftmat): out[m] = rhs[m-1]
            shiftmat = cpool.tile([P, P], F32)
            nc.gpsimd.memset(shiftmat[:], 0.0)
            nc.gpsimd.affine_select(
                out=shiftmat[:], in_=shiftmat[:],
                compare_op=mybir.AluOpType.not_equal,
                fill=1.0, base=1, pattern=[[-1, P]], channel_multiplier=1,
            )

            # ---------------- encoder stream (SWDGE FIFO: in batch order) -------------
            # batch 7 split into two halves, second half first, to shorten the
            # last-arriving dependency chain.
            enc_sb = []
            w_nats = {}
            ipw_nat = None
            for b in range(BL):
                t = epool.tile([P, KC, H], BF16, tag="enc", name=f"enc_sb{b}")
                if b == BL - 1:
                    esrc = enc[b].rearrange("(p k) h -> p k h", k=KC)
                    nc.gpsimd.dma_start(t[:, KC // 2 :, :], esrc[:, KC // 2 :, :])
                    nc.gpsimd.dma_start(t[:, : KC // 2, :], esrc[:, : KC // 2, :])
                else:
                    nc.gpsimd.dma_start(t[:], enc[b].rearrange("(p k) h -> p k h", k=KC))
                enc_sb.append(t)
                if b == 1:
                    # weight loads ride the SWDGE FIFO behind the first two batches
                    for name, param in (
                        ("ih0", W_ih[0]), ("hh0", W_hh[0]),
                        ("ih1", W_ih[1]), ("hh1", W_hh[1]),
                    ):
                        nat = wpool.tile([P, 4, H], BF16, tag=f"wnat_{name}", bufs=1, name=f"wnat_{name}")
                        nc.gpsimd.dma_start(nat[:], param.rearrange("(g p) c -> p g c", p=P))
                        w_nats[name] = nat
                    ipw_nat = wpool.tile([P, 2, H], BF16, tag="wnat_ipw", bufs=1)
                    nc.gpsimd.dma_start(ipw_nat[:], inp_W.rearrange("m (j c) -> m j c", c=H))

            # ---------------- small loads (HWDGE sync queue; early FIFO slots) -------
            ones_row = cpool.tile([1, P], F32)
            nc.vector.memset(ones_row[:], 1.0)
            ones_col = cpool.tile([P, 1], F32)
            nc.vector.memset(ones_col[:], 1.0)
            onesb_row = cpool.tile([1, BL], BF16)
            nc.vector.memset(onesb_row[:], 1.0)

            # pre-warm ACT function tables so no table load lands mid-pipeline
            warm = cpool.tile([1, 1], F32)
            nc.scalar.activation(warm[:], ones_row[0:1, 0:1], AF.Exp)
            nc.scalar.activation(warm[:], ones_row[0:1, 0:1], AF.Sigmoid)
            nc.scalar.activation(warm[:], ones_row[0:1, 0:1], AF.Tanh)

            attW_row = cpool.tile([1, 2 * H], F32)
            nc.sync.dma_start(attW_row[:], att_W[:])
            attb_t = cpool.tile([1, 1], F32)
            nc.sync.dma_start(attb_t[:], att_b[:])
            h0_nat = cpool.tile([BL, 2, H], F32)
            nc.sync.dma_start(h0_nat[:], h0.rearrange("l b h -> b l h"))
            c0_nat = cpool.tile([BL, 2, H], F32)
            nc.sync.dma_start(c0_nat[:], c0.rearrange("l b h -> b l h"))
            in_nat = cpool.tile([BL, I], F32)
            nc.sync.dma_start(in_nat[:], inp[:, 0, :])
            b_rows = []
            for l in range(2):
                bi = cpool.tile([1, 4 * H], F32, tag=f"bi{l}")
                nc.sync.dma_start(bi[:], b_ih[l][:])
                bh = cpool.tile([1, 4 * H], F32, tag=f"bh{l}")
                nc.sync.dma_start(bh[:], b_hh[l][:])
                b_rows.append((bi, bh))
            ipb_row = cpool.tile([1, I], F32)
            nc.sync.dma_start(ipb_row[:], inp_b[:])

            # ---------------- early prep: attention bias, state transposes ----------
            identity_bf = cpool.tile([P, P], BF16)
            nc.scalar.copy(identity_bf[:], identity[:])

            wa_ps = ppool.tile([P, H], F32, tag="ps")
            nc.tensor.matmul(wa_ps[:], ones_row[:], attW_row[:, 0:H], start=True, stop=True)
            wa_bc = cpool.tile([P, H], BF16)
            nc.scalar.copy(wa_bc[:], wa_ps[:])

            wst_ps = ppool.tile([P, 1], F32, tag="ps")
            nc.tensor.matmul(wst_ps[:], attW_row[:, H : 2 * H], ones_col[0:1, :], start=True, stop=True)
            wst_col = cpool.tile([P, 1], F32)
            nc.scalar.copy(wst_col[:], wst_ps[:])

            h0T = cpool.tile([P, 2, BL], F32)
            h0Tb = cpool.tile([P, 2, BL], BF16)
            c0T = cpool.tile([P, 2, BL], F32)
            for l in range(2):
                pt = ppool.tile([P, BL], F32, tag="ps")
                nc.tensor.transpose(pt[:], h0_nat[:, l, :], identity[0:BL, 0:BL])
                nc.scalar.copy(h0T[:, l, :], pt[:])
                nc.scalar.copy(h0Tb[:, l, :], pt[:])
                pt2 = ppool.tile([P, BL], F32, tag="ps")
                nc.tensor.transpose(pt2[:], c0_nat[:, l, :], identity[0:BL, 0:BL])
                nc.scalar.copy(c0T[:, l, :], pt2[:])
            inT_ps = ppool.tile([P, BL], F32, tag="ps")
            nc.tensor.transpose(inT_ps[:], in_nat[:], identity[0:BL, 0:BL])
            inputT = cpool.tile([P, BL], BF16)
            nc.scalar.copy(inputT[:], inT_ps[:])

            astT = cpool.tile([P, BL], F32)
            nc.vector.tensor_tensor(astT[:], h0T[:, 1, :], c0T[:, 1, :], MUL)
            sb_ps = ppool.tile([1, BL], F32, tag="ps")
            nc.tensor.matmul(sb_ps[:], wst_col[:], astT[:], start=True, stop=True)
            sb_row = cpool.tile([1, BL], F32)
            nc.scalar.activation(sb_row[:], sb_ps[:], AF.Identity, bias=attb_t[0:1, 0:1])
            ba_ps = ppool.tile([P, BL], F32, tag="ps")
            nc.tensor.matmul(ba_ps[:], ones_row[:], sb_row[:], start=True, stop=True)
            bias_all = cpool.tile([P, BL], F32)
            nc.scalar.copy(bias_all[:], ba_ps[:])

            # ---------------- weight prep (weights arrive early via SWDGE head) -------
            wT = {}
            for name, nat in w_nats.items():
                dst = cpool.tile([P, 4, H], BF16, tag=f"wT_{name}", name=f"wT_{name}")
                for g in range(4):
                    pt = ppool.tile([P, P], BF16, tag="psb", bufs=2)
                    nc.tensor.transpose(pt[:], nat[:, g, :], identity_bf[:])
                    nc.scalar.copy(dst[:, g, :], pt[:])
                wT[name] = dst
            inp_WT = cpool.tile([P, 2, H], BF16)
            for j in range(2):
                pt = ppool.tile([P, P], BF16, tag="psb", bufs=2)
                nc.tensor.transpose(pt[:], ipw_nat[:, j, :], identity_bf[:])
                nc.scalar.copy(inp_WT[:, j, :], pt[:])

            bsum_rows = []
            for l in range(2):
                bi, bh = b_rows[l]
                bsum = cpool.tile([1, 4 * H], BF16, tag=f"bsum{l}", name=f"bsum{l}")
                nc.vector.tensor_tensor(bsum[:], bi[:], bh[:], ADD)
                bsum_rows.append(bsum)
            ipb_ps = ppool.tile([P, 1], F32, tag="ps")
            nc.tensor.matmul(ipb_ps[:], ipb_row[:], ones_col[0:1, :], start=True, stop=True)
            ipb_col = cpool.tile([P, 1], F32)
            nc.scalar.copy(ipb_col[:], ipb_ps[:])

            # ---------------- main loop over local batches ----------------
            xT_sb = cpool.tile([P, BL], BF16)

            def score_half(b, tag, k0, k1):
                """scores for word columns [k0, k1) of batch b (f32)."""
                kw = k1 - k0
                tmp = wpool.tile([P, kw, H], BF16, tag=f"tmp{tag}", name=f"tmp{tag}_{b}")
                nc.vector.tensor_tensor(
                    tmp[:], enc_sb[b][:, k0:k1, :],
                    wa_bc[:, None, :].to_broadcast((P, kw, H)), MUL,
                )
                t1 = wpool.tile([P, kw, 64], BF16, tag=f"t1{tag}", name=f"t1{tag}_{b}")
                nc.vector.tensor_tensor(t1[:], tmp[:, :, 0:64], tmp[:, :, 64:128], ADD)
                t2 = wpool.tile([P, kw, 32], BF16, tag=f"t2{tag}", name=f"t2{tag}_{b}")
                nc.vector.tensor_tensor(t2[:], t1[:, :, 0:32], t1[:, :, 32:64], ADD)
                t3 = wpool.tile([P, kw, 16], BF16, tag=f"t3{tag}", name=f"t3{tag}_{b}")
                nc.vector.tensor_tensor(t3[:], t2[:, :, 0:16], t2[:, :, 16:32], ADD)
                s_out = spool.tile([P, kw], F32, tag=f"s{tag}", name=f"s{tag}_{b}")
                nc.vector.tensor_reduce(s_out[:], t3[:], mybir.AxisListType.X, ADD)
                return s_out

            for b in range(BL):
                e_sb = spool.tile([P, KC], BF16, tag="e", name=f"e_sb{b}")
                bias_col = bias_all[:, b : b + 1]

                if b == BL - 1:
                    # halves in DMA-arrival order: k 16..31 first
                    s_hi = score_half(b, "h2", KC // 2, KC)
                    bnd_ps = ppool.tile([P, 1], F32, tag="ps")
                    nc.tensor.matmul(bnd_ps[:], shiftmat[:], s_hi[:, -1:], start=True, stop=True)
                    acc1 = spool.tile([P, 1], F32, tag="acc1")
                    acc2 = spool.tile([P, 1], F32, tag="acc2")
                    nc.scalar.activation(
                        e_sb[:, KC // 2 + 1 : KC], s_hi[:, 0 : KC // 2 - 1], AF.Exp, bias=bias_col,
                        accum_out=acc2[:],
                    )
                    nc.scalar.activation(e_sb[:, 0:1], bnd_ps[:], AF.Exp, bias=bias_col)
                    s_lo = score_half(b, "h1", 0, KC // 2)
                    nc.scalar.activation(
                        e_sb[:, 1 : KC // 2 + 1], s_lo[:], AF.Exp, bias=bias_col,
                        accum_out=acc1[:],
                    )
                else:
                    s_sb = score_half(b, "f", 0, KC)
                    bnd_ps = ppool.tile([P, 1], F32, tag="ps")
                    nc.tensor.matmul(bnd_ps[:], shiftmat[:], s_sb[:, KC - 1 : KC], start=True, stop=True)
                    acc1 = spool.tile([P, 1], F32, tag="acc1")
                    nc.scalar.activation(
                        e_sb[:, 1:KC], s_sb[:, 0 : KC - 1], AF.Exp, bias=bias_col,
                        accum_out=acc1[:],
                    )
                    nc.scalar.activation(e_sb[:, 0:1], bnd_ps[:], AF.Exp, bias=bias_col)
                nc.vector.memset(e_sb[0:1, 0:1], 1.0)

                # Z = sum(e): accumulated exp sums + the word-0 column
                esum = spool.tile([P, 1], F32, tag="esum")
                if b == BL - 1:
                    nc.vector.tensor_tensor(esum[:], acc1[:], acc2[:], ADD)
                    nc.vector.tensor_tensor(esum[:], esum[:], e_sb[:, 0:1], ADD)
                else:
                    nc.vector.tensor_tensor(esum[:], acc1[:], e_sb[:, 0:1], ADD)
                z_ps = ppool.tile([1, 1], F32, tag="ps")
                nc.tensor.matmul(z_ps[:], esum[:], ones_col[:], start=True, stop=True)
                rz = spool.tile([1, 1], F32, tag="rz")
                nc.vector.reciprocal(rz[:], z_ps[:])
                rz_ps = ppool.tile([P, 1], F32, tag="ps")
                nc.tensor.matmul(rz_ps[:], ones_row[:], rz[:], start=True, stop=True)
                rz_col = spool.tile([P, 1], F32, tag="rzc")
                nc.scalar.copy(rz_col[:], rz_ps[:])

                # weighted sum: x row = sum_k e_k.T @ enc_chunk_k (e stationary)
                xrow_ps = pxpool.tile([1, P], F32, tag="xr")
                if b == BL - 1:
                    korder = list(range(KC // 2 + 1, KC)) + list(range(0, KC // 2 + 1))
                else:
                    korder = list(range(KC))
                for j, k in enumerate(korder):
                    nc.tensor.matmul(
                        xrow_ps[:], e_sb[:, k : k + 1], enc_sb[b][:, k, :],
                        start=(j == 0), stop=(j == KC - 1),
                    )
                x_row = spool.tile([1, P], F32, tag="xrow")
                nc.scalar.copy(x_row[:], xrow_ps[:])
                xcol = pxpool.tile([P, 1], F32, tag="xc", bufs=1)
                nc.tensor.matmul(xcol[:], x_row[:], ones_col[0:1, :], start=True, stop=True)
                nc.scalar.activation(
                    xT_sb[:, b : b + 1], xcol[:], AF.Copy, scale=rz_col[:]
                )

            # ---------------- tail: input MLP + 2-layer LSTM (bf16 matmuls) ---------
            xin_ps = ppool.tile([P, BL], F32, tag="ps")
            nc.tensor.matmul(xin_ps[:], inp_WT[:, 0, :], xT_sb[:], start=True, stop=False)
            nc.tensor.matmul(xin_ps[:], inp_WT[:, 1, :], inputT[:], start=False, stop=True)
            xinT = cpool.tile([P, BL], BF16)
            nc.scalar.activation(xinT[:], xin_ps[:], AF.Identity, bias=ipb_col[:])

            hcT = cpool.tile([P, 4 * BL], F32)  # cols: h1 | h2 | c1 | c2
            h1Tb = cpool.tile([P, BL], BF16)

            # psum gate column order [i, f, o, g]; weight row-chunks are (i,f,g,o)
            GATE_ORDER = (0, 1, 3, 2)
            for l in range(2):
                x_rhs = xinT[:] if l == 0 else h1Tb[:]
                wih, whh = wT[f"ih{l}"], wT[f"hh{l}"]
                bsum = bsum_rows[l]
                g_ps = ppool.tile([P, 4 * BL], F32, tag="ps")
                for col, g in enumerate(GATE_ORDER):
                    sl = g_ps[:, col * BL : (col + 1) * BL]
                    nc.tensor.matmul(sl, wih[:, g, :], x_rhs, start=True, stop=False)
                    nc.tensor.matmul(sl, whh[:, g, :], h0Tb[:, l, :], start=False, stop=False)
                    nc.tensor.matmul(
                        sl, bsum[:, g * H : (g + 1) * H], onesb_row[:], start=False, stop=True
                    )
                sig = spool.tile([P, 3 * BL], F32, tag="sig")
                nc.scalar.activation(sig[:], g_ps[:, 0 : 3 * BL], AF.Sigmoid)
                tng = spool.tile([P, BL], F32, tag="tng")
                nc.scalar.activation(tng[:], g_ps[:, 3 * BL : 4 * BL], AF.Tanh)

                fc = spool.tile([P, BL], F32, tag="fc")
                nc.vector.tensor_tensor(fc[:], sig[:, BL : 2 * BL], c0T[:, l, :], MUL)
                ig = spool.tile([P, BL], F32, tag="ig")
                nc.vector.tensor_tensor(ig[:], sig[:, 0:BL], tng[:], MUL)
                c_new = hcT[:, (2 + l) * BL : (3 + l) * BL]
                nc.vector.tensor_tensor(c_new, fc[:], ig[:], ADD)
                tc_t = spool.tile([P, BL], F32, tag="tc")
                nc.scalar.activation(tc_t[:], c_new, AF.Tanh)
                h_new = hcT[:, l * BL : (l + 1) * BL]
                nc.vector.tensor_tensor(h_new, sig[:, 2 * BL : 3 * BL], tc_t[:], MUL)
                if l == 0:
                    nc.vector.tensor_copy(h1Tb[:], h_new)

            outT_ps = ppool.tile([4 * BL, P], F32, tag="ps")
            nc.tensor.transpose(outT_ps[:], hcT[:], identity[:])
            out_stage = cpool.tile([4 * BL, P], F32)
            nc.scalar.copy(out_stage[:], outT_ps[:])
            nc.sync.dma_start(out.rearrange("j b h -> (j b) h"), out_stage[:])

    nc.compile()
    return nc


_NC_CACHE = {}


def _get_nc():
    if "nc" not in _NC_CACHE:
        _NC_CACHE["nc"] = build_bass(debug=False)
    return _NC_CACHE["nc"]


def _shard_inputs(inputs: dict) -> list[dict]:
    in_maps = []
    for i in range(N_CORES):
        sl = slice(i * BL, (i + 1) * BL)
        m = {
            "input": np.ascontiguousarray(inputs["input"][sl]),
            "h0": np.ascontiguousarray(inputs["h0"][:, sl]),
            "c0": np.ascontiguousarray(inputs["c0"][:, sl]),
            "encoder_output": np.ascontiguousarray(inputs["encoder_output"][sl]),
        }
        for k in (
            "att_W", "att_b", "inp_W", "inp_b",
            "W_ih0", "W_hh0", "b_ih0", "b_hh0",
            "W_ih1", "W_hh1", "b_ih1", "b_hh1",
        ):
            m[k] = np.ascontiguousarray(inputs[k])
        in_maps.append(m)
    return in_maps


def run_spmd(inputs: dict, trace: bool = False):
    """Returns (outputs_tuple, exec_time_ns_or_None)."""
    nc = _get_nc()
    in_maps = _shard_inputs(inputs)
    res = run_bass_kernel_spmd(nc, in_maps, list(range(N_CORES)), trace=trace)
    packed = np.concatenate([res.results[i]["out"] for i in range(N_CORES)], axis=1)
    h1, h2, c1, c2 = packed[0], packed[1], packed[2], packed[3]
    output = h2[:, None, :].astype(np.float32)
    h_stack = np.stack([h1, h2]).astype(np.float32)
    c_stack = np.stack([c1, c2]).astype(np.float32)
    return (output, h_stack, c_stack), res.exec_time_ns


def kernel(**inputs):
    inputs = {k: np.asarray(v, dtype=np.float32) for k, v in inputs.items()}
    outs, _ = run_spmd(inputs, trace=False)
    return outs
